# revision 15
# baseline (speedup 1.0000x reference)
"""Trainium2 Bass kernel for nn_AgentTwo (ragged-sequence GRU agent).

Full-input contract: kernel(**inputs) takes the unsharded numpy inputs and
returns the full [8192, 10] float32 action probabilities.

Strategy (pure data parallel over 8 NeuronCores, B=8192 -> 1024 rows/core):
 - Host resolves the ragged aliveness up front: per row, tokens at steps at
   or after the first zero are rewritten to a sentinel embedding row, solved
   on host so the z-gate pre-activation saturates sigmoid (zbar == 0),
   freezing h exactly on device -- the reference's "output_state while
   alive" semantics fall out with zero extra device work.
 - Host resolves the embedding lookup: the per-core bf16 stream carries
   [emb(tok) | emb(tok)@W_ihn.T + b_ihn] in [E, B] layout (E on
   partitions), so the device reads embedding bytes as plain sequential
   DMA (full HBM efficiency, no per-row descriptor generation).
 - Device per step t (layout [gate/hidden=128 partitions, batch free],
   two independent 512-column streams so the recurrence chains pipeline):
     psum_r  = Wihr @ embT + Whhr @ hT          (PE, bf16 in / f32 acc)
     psum_zn = -Wihz @ embT - Whhz @ hT
     psum_hn = Whhn @ hT
     r    = sigmoid(psum_r + b_r)               (ACT, bias fused)
     zbar = sigmoid(psum_zn - b_z)              (ACT, bias fused)
     tg   = (psum_hn + b_hhn) * r               (DVE scalar_tensor_tensor)
     npre = tg + gi_nT                          (DVE)
     n    = tanh(npre)                          (ACT)
     h'   = h + zbar * (n - h)                  (DVE x3, bf16)
 - Head: logitsT = w_out @ h (PE), expv = exp(logitsT + b_out) (ACT); host
   normalizes the softmax in f64 and reassembles [8192, 10].
"""

import sys

for _p in ("/opt/trn_rl_repo",):
    if _p not in sys.path:
        sys.path.append(_p)

import numpy as np
import ml_dtypes

import concourse.bass as bass
import concourse.mybir as mybir
import concourse.tile as tile
from concourse import bacc
from concourse.bass_utils import run_bass_kernel_spmd

BF16 = ml_dtypes.bfloat16

NCORES = 8
B, T, E, V, A = 8192, 64, 128, 32000, 10
V1 = V + 1          # vocab rows (0..32000)
BL = B // NCORES    # 1024 rows per core
HALF = BL // 2      # 512-column stream width
GS = 2              # timesteps per stream DMA
NG = T // GS        # stream groups
F32 = mybir.dt.float32
BF = mybir.dt.bfloat16

_CACHE = {}


def _build_nc(T=T, BL=BL, NG=NG):
    HALF = BL // 2
    nc = bacc.Bacc(None)
    es_d = nc.declare_dram_parameter("estream", [NG, 128, GS * 2 * BL], BF, isOutput=False)
    w_d = nc.declare_dram_parameter("wstat", [128, 6 * E], BF, isOutput=False)
    bias_d = nc.declare_dram_parameter("biasp", [128, 3], F32, isOutput=False)
    wout_d = nc.declare_dram_parameter("woutT", [128, A], BF, isOutput=False)
    bout_d = nc.declare_dram_parameter("bout", [A, 1], F32, isOutput=False)
    out_d = nc.declare_dram_parameter("expv", [A, BL], F32, isOutput=True)

    SIG = mybir.ActivationFunctionType.Sigmoid
    TANH = mybir.ActivationFunctionType.Tanh
    EXP = mybir.ActivationFunctionType.Exp
    ADD = mybir.AluOpType.add
    MULT = mybir.AluOpType.mult

    with tile.TileContext(nc) as tc:
        with (
            tc.tile_pool(name="const", bufs=1) as cp,
            tc.tile_pool(name="gath", bufs=6) as gathp,
            tc.tile_pool(name="hA", bufs=4) as hpA,
            tc.tile_pool(name="hB", bufs=4) as hpB,
            tc.tile_pool(name="gates", bufs=4) as gp,
            tc.tile_pool(name="ps", bufs=1, space=bass.MemorySpace.PSUM) as psp,
        ):
            w_sb = cp.tile([128, 6 * E], BF, tag="w")
            bias_sb = cp.tile([128, 3], F32, tag="bias")
            wout_sb = cp.tile([128, A], BF, tag="wout")
            bout_sb = cp.tile([A, 1], F32, tag="bout")
            nc.sync.dma_start(w_sb[:], w_d[:])
            nc.sync.dma_start(bias_sb[:], bias_d[:])
            nc.sync.dma_start(wout_sb[:], wout_d[:])
            nc.sync.dma_start(bout_sb[:], bout_d[:])

            # weight column slices in w_sb: [ihr | -ihz | hhr | -hhz | hhn]
            W_IHR = w_sb[:, 0 * E:1 * E]
            W_IHZN = w_sb[:, 1 * E:2 * E]
            W_HHR = w_sb[:, 2 * E:3 * E]
            W_HHZN = w_sb[:, 3 * E:4 * E]
            W_HHN = w_sb[:, 4 * E:5 * E]
            W_I = w_sb[:, 5 * E:6 * E]
            B_R = bias_sb[:, 0:1]
            B_ZN = bias_sb[:, 1:2]   # -(b_ihz + b_hhz)
            B_HHN = bias_sb[:, 2:3]

            h_cur = []
            for s, hp in ((0, hpA), (1, hpB)):
                h0 = hp.tile([128, HALF], BF, tag=f"h{s}")
                nc.vector.memset(h0[:], 0.0)
                h_cur.append(h0)

            for g in range(NG):
                ep = gathp.tile([128, GS, 2, BL], BF, tag="ep")
                nc.sync.dma_start(ep[:], es_d[g])
                for k in range(GS):
                    for s in range(2):
                        lo = s * HALF
                        hi = lo + HALF
                        embT = ep[:, k, 0, lo:hi]
                        pnT = ep[:, k, 1, lo:hi]
                        h = h_cur[s]

                        ps_r = psp.tile([128, HALF], F32, tag=f"r{s}")
                        ps_z = psp.tile([128, HALF], F32, tag=f"z{s}")
                        ps_hn = psp.tile([128, HALF], F32, tag=f"hn{s}")
                        # input-side projections first: they only need the
                        # stream, so PE can run them while waiting for h'
                        nc.tensor.matmul(ps_r[:], W_IHR, embT, start=True, stop=False)
                        nc.tensor.matmul(ps_z[:], W_IHZN, embT, start=True, stop=False)
                        nc.tensor.matmul(ps_r[:], W_HHR, h[:], start=False, stop=True)
                        nc.tensor.matmul(ps_hn[:], W_HHN, h[:], start=True, stop=True)
                        nc.tensor.matmul(ps_z[:], W_HHZN, h[:], start=False, stop=True)

                        r = gp.tile([128, HALF], BF, tag=f"r{s}")
                        zb = gp.tile([128, HALF], BF, tag=f"zb{s}")
                        tg = gp.tile([128, HALF], BF, tag=f"tg{s}")
                        npre = gp.tile([128, HALF], BF, tag=f"np{s}")
                        n = gp.tile([128, HALF], BF, tag=f"n{s}")
                        d = gp.tile([128, HALF], BF, tag=f"d{s}")
                        e = gp.tile([128, HALF], BF, tag=f"e{s}")
                        hn2 = (hpA if s == 0 else hpB).tile([128, HALF], BF, tag=f"h{s}")

                        nc.scalar.activation(r[:], ps_r[:], SIG, bias=B_R)
                        nc.scalar.activation(zb[:], ps_z[:], SIG, bias=B_ZN)
                        nc.vector.scalar_tensor_tensor(tg[:], ps_hn[:], B_HHN, r[:], ADD, MULT)
                        nc.tensor.matmul(ps_hn[:], W_I, tg[:], start=True, stop=False)
                        nc.tensor.matmul(ps_hn[:], W_I, pnT, start=False, stop=True)
                        nc.scalar.activation(n[:], ps_hn[:], TANH)
                        nc.vector.tensor_sub(d[:], n[:], h[:])
                        nc.vector.tensor_mul(e[:], zb[:], d[:])
                        nc.vector.tensor_add(hn2[:], h[:], e[:])
                        h_cur[s] = hn2

            ps_l = psp.tile([A, BL], F32, tag="logits")
            nc.tensor.matmul(ps_l[:, 0:HALF], wout_sb[:], h_cur[0][:], start=True, stop=True)
            nc.tensor.matmul(ps_l[:, HALF:BL], wout_sb[:], h_cur[1][:], start=True, stop=True)
            expv = cp.tile([A, BL], F32, tag="expv")
            nc.scalar.activation(expv[:], ps_l[:], EXP, bias=bout_sb[:, 0:1])
            nc.sync.dma_start(out_d[:], expv[:])

    nc.finalize()
    return nc


def _prep_host(utterance, emb_table, w_ih, w_hh, b_ih, b_hh, w_out, b_out):
    utt = np.asarray(utterance).astype(np.int64)
    emb = np.asarray(emb_table).astype(np.float32)
    w_ih = np.asarray(w_ih).astype(np.float32)
    w_hh = np.asarray(w_hh).astype(np.float32)
    b_ih = np.asarray(b_ih).astype(np.float32)
    b_hh = np.asarray(b_hh).astype(np.float32)
    w_out = np.asarray(w_out).astype(np.float32)
    b_out = np.asarray(b_out).astype(np.float32)

    # --- sentinel embedding: saturate the z gate for dead rows.  The z
    # weights are negated on device, so we need W_ihz @ v large POSITIVE
    # (zbar = sigmoid(-(i_z + h_z + b_z)) -> 0).
    W_ihz = w_ih[E:2 * E].astype(np.float64)
    W_hhz = w_hh[E:2 * E]
    b_z = b_ih[E:2 * E] + b_hh[E:2 * E]
    bound = np.abs(W_hhz).sum(axis=1) + np.abs(b_z)
    margin = 0.0
    slack = 120.0
    for _ in range(6):
        v = np.linalg.solve(W_ihz, (bound + slack).astype(np.float64))
        v_bf = v.astype(BF16).astype(np.float32)
        zpre = w_ih[E:2 * E].astype(BF16).astype(np.float32) @ v_bf
        margin = float((zpre - bound).min())
        if margin >= 25.0:
            break
        slack *= 2.0
    assert margin >= 25.0, f"sentinel margin too small: {margin}"

    # --- death-step index rewrite ---
    nz = utt != 0                                  # [B, T]
    alive0 = np.ones((B, 1), bool)
    alive_t = np.concatenate([alive0, np.cumprod(nz[:, :-1], axis=1).astype(bool)], axis=1)
    idx = np.where(alive_t, utt, V1).astype(np.int32)     # [B, T]

    # --- combined table [emb | proj_n] bf16 (+ sentinel row) ---
    proj_n = emb @ w_ih[2 * E:3 * E].T + b_ih[2 * E:3 * E]
    table = np.zeros((V1 + 1, 2, E), BF16)
    table[:V1, 0] = emb.astype(BF16)
    table[:V1, 1] = proj_n.astype(BF16)
    table[V1, 0] = v_bf.astype(BF16)
    table_u16 = table.view(np.uint16)              # [V1+1, 2, E]

    # --- dense per-core embedding stream [NG, 128, GS*2*BL] bf16 ---
    streams = []
    for cix in range(NCORES):
        ids = idx[cix * BL:(cix + 1) * BL]         # [BL, T]
        gat = table_u16[ids]                       # [BL, T, 2, E] u16
        gat = gat.reshape(BL, NG, GS, 2, E)
        st = np.ascontiguousarray(np.transpose(gat, (1, 4, 2, 3, 0)))  # [NG, E, GS, 2, BL]
        streams.append(st.reshape(NG, 128, GS * 2 * BL).view(BF16))

    wstat = np.concatenate(
        [w_ih[0:E].T, -w_ih[E:2 * E].T, w_hh[0:E].T, -w_hh[E:2 * E].T, w_hh[2 * E:3 * E].T,
         np.eye(E, dtype=np.float32)],
        axis=1,
    ).astype(BF16)                                  # [128, 768]
    biasp = np.stack(
        [b_ih[0:E] + b_hh[0:E], -(b_ih[E:2 * E] + b_hh[E:2 * E]), b_hh[2 * E:3 * E]],
        axis=1,
    ).astype(np.float32)                            # [128, 3]
    woutT = np.ascontiguousarray(w_out.T).astype(BF16)   # [128, 10]
    bout = b_out.reshape(A, 1).astype(np.float32)

    shared = {"wstat": wstat, "biasp": biasp, "woutT": woutT, "bout": bout}
    return [dict(shared, estream=streams[c]) for c in range(NCORES)]


def kernel(utterance, global_idxes, emb_table, w_ih, w_hh, b_ih, b_hh, w_out, b_out):
    in_maps = _prep_host(utterance, emb_table, w_ih, w_hh, b_ih, b_hh, w_out, b_out)
    if "nc" not in _CACHE:
        _CACHE["nc"] = _build_nc()
    nc = _CACHE["nc"]
    res = run_bass_kernel_spmd(nc, in_maps, core_ids=list(range(NCORES)))
    out = np.empty((B, A), np.float64)
    for c in range(NCORES):
        expv = res.results[c]["expv"].astype(np.float64)       # [A, BL]
        out[c * BL:(c + 1) * BL] = (expv / expv.sum(axis=0, keepdims=True)).T
    return out.astype(np.float32)


# revision 16
# speedup vs baseline: 1.0112x; 1.0112x over previous
"""Trainium2 Bass kernel for nn_AgentTwo (ragged-sequence GRU agent).

Full-input contract: kernel(**inputs) takes the unsharded numpy inputs and
returns the full [8192, 10] float32 action probabilities.

Strategy (pure data parallel over 8 NeuronCores, B=8192 -> 1024 rows/core):
 - Host resolves the ragged aliveness up front: per row, tokens at steps at
   or after the first zero are rewritten to a sentinel embedding row, solved
   on host so the z-gate pre-activation saturates sigmoid (zbar == 0),
   freezing h exactly on device -- the reference's "output_state while
   alive" semantics fall out with zero extra device work.
 - Host resolves the embedding lookup: the per-core bf16 stream carries
   [emb(tok) | emb(tok)@W_ihn.T + b_ihn] in [E, B] layout (E on
   partitions), so the device reads embedding bytes as plain sequential
   DMA (full HBM efficiency, no per-row descriptor generation).
 - Device per step t (layout [gate/hidden=128 partitions, batch free],
   two independent 512-column streams so the recurrence chains pipeline):
     psum_r  = Wihr @ embT + Whhr @ hT          (PE, bf16 in / f32 acc)
     psum_zn = -Wihz @ embT - Whhz @ hT
     psum_hn = Whhn @ hT
     r    = sigmoid(psum_r + b_r)               (ACT, bias fused)
     zbar = sigmoid(psum_zn - b_z)              (ACT, bias fused)
     tg   = (psum_hn + b_hhn) * r               (DVE scalar_tensor_tensor)
     npre = tg + gi_nT                          (DVE)
     n    = tanh(npre)                          (ACT)
     h'   = h + zbar * (n - h)                  (DVE x3, bf16)
 - Head: logitsT = w_out @ h (PE), expv = exp(logitsT + b_out) (ACT); host
   normalizes the softmax in f64 and reassembles [8192, 10].
"""

import sys

for _p in ("/opt/trn_rl_repo",):
    if _p not in sys.path:
        sys.path.append(_p)

import numpy as np
import ml_dtypes

import concourse.bass as bass
import concourse.mybir as mybir
import concourse.tile as tile
from concourse import bacc
from concourse.bass_utils import run_bass_kernel_spmd

BF16 = ml_dtypes.bfloat16

NCORES = 8
B, T, E, V, A = 8192, 64, 128, 32000, 10
V1 = V + 1          # vocab rows (0..32000)
BL = B // NCORES    # 1024 rows per core
HALF = BL // 2      # 512-column stream width
GS = 2              # timesteps per stream DMA
NG = T // GS        # stream groups
F32 = mybir.dt.float32
BF = mybir.dt.bfloat16

_CACHE = {}


def _build_nc(T=T, BL=BL, NG=NG):
    HALF = BL // 2
    nc = bacc.Bacc(None)
    es_d = nc.declare_dram_parameter("estream", [NG, 128, GS * 2 * BL], BF, isOutput=False)
    w_d = nc.declare_dram_parameter("wstat", [128, 6 * E], BF, isOutput=False)
    bias_d = nc.declare_dram_parameter("biasp", [128, 3], F32, isOutput=False)
    wout_d = nc.declare_dram_parameter("woutT", [128, A], BF, isOutput=False)
    bout_d = nc.declare_dram_parameter("bout", [A, 1], F32, isOutput=False)
    out_d = nc.declare_dram_parameter("expv", [A, BL], F32, isOutput=True)

    SIG = mybir.ActivationFunctionType.Sigmoid
    TANH = mybir.ActivationFunctionType.Tanh
    EXP = mybir.ActivationFunctionType.Exp
    ADD = mybir.AluOpType.add
    MULT = mybir.AluOpType.mult

    with tile.TileContext(nc) as tc:
        with (
            tc.tile_pool(name="const", bufs=1) as cp,
            tc.tile_pool(name="gath", bufs=6) as gathp,
            tc.tile_pool(name="hA", bufs=4) as hpA,
            tc.tile_pool(name="hB", bufs=4) as hpB,
            tc.tile_pool(name="gates", bufs=4) as gp,
            tc.tile_pool(name="ps", bufs=1, space=bass.MemorySpace.PSUM) as psp,
        ):
            w_sb = cp.tile([128, 6 * E], BF, tag="w")
            bias_sb = cp.tile([128, 3], F32, tag="bias")
            wout_sb = cp.tile([128, A], BF, tag="wout")
            bout_sb = cp.tile([A, 1], F32, tag="bout")
            nc.sync.dma_start(w_sb[:], w_d[:])
            nc.sync.dma_start(bias_sb[:], bias_d[:])
            nc.sync.dma_start(wout_sb[:], wout_d[:])
            nc.sync.dma_start(bout_sb[:], bout_d[:])

            # weight column slices in w_sb: [ihr | -ihz | hhr | -hhz | hhn]
            W_IHR = w_sb[:, 0 * E:1 * E]
            W_IHZN = w_sb[:, 1 * E:2 * E]
            W_HHR = w_sb[:, 2 * E:3 * E]
            W_HHZN = w_sb[:, 3 * E:4 * E]
            W_HHN = w_sb[:, 4 * E:5 * E]
            W_I = w_sb[:, 5 * E:6 * E]
            B_R = bias_sb[:, 0:1]
            B_ZN = bias_sb[:, 1:2]   # -(b_ihz + b_hhz)
            B_HHN = bias_sb[:, 2:3]

            h_cur = []
            for s, hp in ((0, hpA), (1, hpB)):
                h0 = hp.tile([128, HALF], BF, tag=f"h{s}")
                nc.vector.memset(h0[:], 0.0)
                h_cur.append(h0)

            for g in range(NG):
                ep = gathp.tile([128, GS, 2, BL], BF, tag="ep")
                nc.sync.dma_start(ep[:], es_d[g])
                for k in range(GS):
                    for s in range(2):
                        lo = s * HALF
                        hi = lo + HALF
                        embT = ep[:, k, 0, lo:hi]
                        pnT = ep[:, k, 1, lo:hi]
                        h = h_cur[s]

                        ps_r = psp.tile([128, HALF], F32, tag=f"r{s}")
                        ps_z = psp.tile([128, HALF], F32, tag=f"z{s}")
                        ps_hn = psp.tile([128, HALF], F32, tag=f"hn{s}")
                        # input-side projections first: they only need the
                        # stream, so PE can run them while waiting for h'
                        nc.tensor.matmul(ps_r[:], W_IHR, embT, start=True, stop=False)
                        nc.tensor.matmul(ps_z[:], W_IHZN, embT, start=True, stop=False)
                        nc.tensor.matmul(ps_r[:], W_HHR, h[:], start=False, stop=True)
                        nc.tensor.matmul(ps_hn[:], W_HHN, h[:], start=True, stop=True)
                        nc.tensor.matmul(ps_z[:], W_HHZN, h[:], start=False, stop=True)

                        r = gp.tile([128, HALF], BF, tag=f"r{s}")
                        zb = gp.tile([128, HALF], BF, tag=f"zb{s}")
                        tg = gp.tile([128, HALF], BF, tag=f"tg{s}")
                        npre = gp.tile([128, HALF], BF, tag=f"np{s}")
                        n = gp.tile([128, HALF], BF, tag=f"n{s}")
                        d = gp.tile([128, HALF], BF, tag=f"d{s}")
                        e = gp.tile([128, HALF], BF, tag=f"e{s}")
                        hn2 = (hpA if s == 0 else hpB).tile([128, HALF], BF, tag=f"h{s}")

                        nc.scalar.activation(r[:], ps_r[:], SIG, bias=B_R)
                        nc.scalar.activation(zb[:], ps_z[:], SIG, bias=B_ZN)
                        nc.vector.scalar_tensor_tensor(tg[:], ps_hn[:], B_HHN, r[:], ADD, MULT)
                        nc.vector.tensor_add(npre[:], tg[:], pnT)
                        nc.scalar.activation(n[:], npre[:], TANH)
                        nc.vector.tensor_sub(d[:], n[:], h[:])
                        nc.vector.tensor_mul(e[:], zb[:], d[:])
                        nc.vector.tensor_add(hn2[:], h[:], e[:])
                        h_cur[s] = hn2

            ps_l = psp.tile([A, BL], F32, tag="logits")
            nc.tensor.matmul(ps_l[:, 0:HALF], wout_sb[:], h_cur[0][:], start=True, stop=True)
            nc.tensor.matmul(ps_l[:, HALF:BL], wout_sb[:], h_cur[1][:], start=True, stop=True)
            expv = cp.tile([A, BL], F32, tag="expv")
            nc.scalar.activation(expv[:], ps_l[:], EXP, bias=bout_sb[:, 0:1])
            nc.sync.dma_start(out_d[:], expv[:])

    nc.finalize()
    return nc


def _prep_host(utterance, emb_table, w_ih, w_hh, b_ih, b_hh, w_out, b_out):
    utt = np.asarray(utterance).astype(np.int64)
    emb = np.asarray(emb_table).astype(np.float32)
    w_ih = np.asarray(w_ih).astype(np.float32)
    w_hh = np.asarray(w_hh).astype(np.float32)
    b_ih = np.asarray(b_ih).astype(np.float32)
    b_hh = np.asarray(b_hh).astype(np.float32)
    w_out = np.asarray(w_out).astype(np.float32)
    b_out = np.asarray(b_out).astype(np.float32)

    # --- sentinel embedding: saturate the z gate for dead rows.  The z
    # weights are negated on device, so we need W_ihz @ v large POSITIVE
    # (zbar = sigmoid(-(i_z + h_z + b_z)) -> 0).
    W_ihz = w_ih[E:2 * E].astype(np.float64)
    W_hhz = w_hh[E:2 * E]
    b_z = b_ih[E:2 * E] + b_hh[E:2 * E]
    bound = np.abs(W_hhz).sum(axis=1) + np.abs(b_z)
    margin = 0.0
    slack = 120.0
    for _ in range(6):
        v = np.linalg.solve(W_ihz, (bound + slack).astype(np.float64))
        v_bf = v.astype(BF16).astype(np.float32)
        zpre = w_ih[E:2 * E].astype(BF16).astype(np.float32) @ v_bf
        margin = float((zpre - bound).min())
        if margin >= 25.0:
            break
        slack *= 2.0
    assert margin >= 25.0, f"sentinel margin too small: {margin}"

    # --- death-step index rewrite ---
    nz = utt != 0                                  # [B, T]
    alive0 = np.ones((B, 1), bool)
    alive_t = np.concatenate([alive0, np.cumprod(nz[:, :-1], axis=1).astype(bool)], axis=1)
    idx = np.where(alive_t, utt, V1).astype(np.int32)     # [B, T]

    # --- combined table [emb | proj_n] bf16 (+ sentinel row) ---
    proj_n = emb @ w_ih[2 * E:3 * E].T + b_ih[2 * E:3 * E]
    table = np.zeros((V1 + 1, 2, E), BF16)
    table[:V1, 0] = emb.astype(BF16)
    table[:V1, 1] = proj_n.astype(BF16)
    table[V1, 0] = v_bf.astype(BF16)
    table_u16 = table.view(np.uint16)              # [V1+1, 2, E]

    # --- dense per-core embedding stream [NG, 128, GS*2*BL] bf16 ---
    streams = []
    for cix in range(NCORES):
        ids = idx[cix * BL:(cix + 1) * BL]         # [BL, T]
        gat = table_u16[ids]                       # [BL, T, 2, E] u16
        gat = gat.reshape(BL, NG, GS, 2, E)
        st = np.ascontiguousarray(np.transpose(gat, (1, 4, 2, 3, 0)))  # [NG, E, GS, 2, BL]
        streams.append(st.reshape(NG, 128, GS * 2 * BL).view(BF16))

    wstat = np.concatenate(
        [w_ih[0:E].T, -w_ih[E:2 * E].T, w_hh[0:E].T, -w_hh[E:2 * E].T, w_hh[2 * E:3 * E].T,
         np.eye(E, dtype=np.float32)],
        axis=1,
    ).astype(BF16)                                  # [128, 768]
    biasp = np.stack(
        [b_ih[0:E] + b_hh[0:E], -(b_ih[E:2 * E] + b_hh[E:2 * E]), b_hh[2 * E:3 * E]],
        axis=1,
    ).astype(np.float32)                            # [128, 3]
    woutT = np.ascontiguousarray(w_out.T).astype(BF16)   # [128, 10]
    bout = b_out.reshape(A, 1).astype(np.float32)

    shared = {"wstat": wstat, "biasp": biasp, "woutT": woutT, "bout": bout}
    return [dict(shared, estream=streams[c]) for c in range(NCORES)]


def kernel(utterance, global_idxes, emb_table, w_ih, w_hh, b_ih, b_hh, w_out, b_out):
    in_maps = _prep_host(utterance, emb_table, w_ih, w_hh, b_ih, b_hh, w_out, b_out)
    if "nc" not in _CACHE:
        _CACHE["nc"] = _build_nc()
    nc = _CACHE["nc"]
    res = run_bass_kernel_spmd(nc, in_maps, core_ids=list(range(NCORES)))
    out = np.empty((B, A), np.float64)
    for c in range(NCORES):
        expv = res.results[c]["expv"].astype(np.float64)       # [A, BL]
        out[c * BL:(c + 1) * BL] = (expv / expv.sum(axis=0, keepdims=True)).T
    return out.astype(np.float32)
